# revision 63
# baseline (speedup 1.0000x reference)
"""Trainium2 Bass kernel for nn_Aspect_Attention_op2 (B=16, L=2048, D=768).

reference semantics:
    y = tanh(x2 @ att_W)                        # [B, L, D]
    wlog = einsum('d,bld->bl', att_v, y)        # [B, L]
    w = softmax(wlog, axis=0)                   # softmax over BATCH
    w_tiled[b,i,j] = w[b, (i*D+j) % L]          # tile-then-reshape (windowed!)
    out = x2 * w_tiled
    score = x @ out^T ; attn = softmax(score, -1) ; ctx = attn @ out

Distribution: batch-parallel, 2 batches/core on 8 cores. The batch softmax
needs one 8KB AllReduce(add) of sum_b exp(wlog) (max-subtraction is skipped:
wlog absmax ~0.4, score absmax ~13; the attention exp carries a -4 bias so
exp(score-4) fits fp16, which cancels in the softmax ratio).

Key structure (all operands fp16, psum fp32; measured rel err ~7e-4):
  * The window multiplier w[(i*D+j) % L] is periodic with period 8 in the
    row index, so `out` never exists in DRAM:
      - natural: out[:,kt,j] = x2[:,kt,j] * wq[p,j] with one fixed [128,768]
        tile wq[p,j] = w[(768*(p%8)+j)%L] for ALL kt, applied in place to
        the SBUF-resident fp16 x2 copy (which carries the PV ones-column
        that yields the softmax denominators).
      - transposed: outT[p,dt,k] = x2T[p,dt,k] * M8[p,dt,k%8] with
        M8[p,dt,r] = w[(768r+128dt+p)%L], applied in place to x2T via a
        stride-0 broadcast along k//8.  M8 itself is a tensor-engine
        transpose of wq's first 8 partitions.
    wq comes from affine DMA reads of a 4x-replicated copy of w in DRAM
    (offsets 768r+j < 8192 need no modulo).
  * x2T is produced by tensor-engine transposes (identity matmul, identity
    shipped as a constant input) straight from the SBUF cast tiles -- no
    DRAM spill / xbar read-back for x2.
  * xT is read back with the DMA xbar transpose from an fp16 spill of x;
    the spill runs off the critical path (batch 0 spread through phase A,
    batch 1 inside batch 0's attention where DMA is idle).
  * q-chunks are processed in PAIRS sharing each stationary tile: the
    second matmul of each pair sets ldweights=False so the PE reuses the
    loaded weights (halves LDWEIGHTS in QK/y/v; PV's two output slices
    share the loaded attn tile the same way).
  * All w/softmax bookkeeping runs in [16,128] column-parallel layout
    (128x faster than single-partition row ops); batch 1's post-AllReduce
    scaling is emitted after batch 0's attention so it cannot block it.

NOTE: gpsimd must run ONLY the collective -- any other gpsimd instruction
ahead of it perturbs the TOPSP doorbell and adds ~2.5ms to the AllReduce.
"""

import sys

try:
    import concourse  # noqa: F401
except ImportError:
    sys.path.insert(0, "/opt/trn_rl_repo")

import numpy as np

import concourse.bass as bass
import concourse.bacc as bacc
import concourse.mybir as mybir
import concourse.tile as tile
from concourse.bass_utils import run_bass_kernel_spmd

B, L, D = 16, 2048, 768
NCORES = 8
NB = B // NCORES          # batches per core = 2
P = 128
DT = D // P               # 6 d-tiles
KT = L // P               # 16 k-tiles
QC = 512                  # q-chunk (psum free dim)
NQC = L // QC             # 4 q-chunks
FP32 = mybir.dt.float32
FP16 = mybir.dt.float16
AF = mybir.ActivationFunctionType
EXP_BIAS = -4.0           # exp(score-4) <= ~5e3 fits fp16; cancels in ratio


def ts(i, n):
    return bass.ts(i, n)


def build_nc():
    nc = bacc.Bacc("TRN2", target_bir_lowering=False, debug=False,
                   num_devices=NCORES)

    x_ext = nc.dram_tensor("x", [NB, L, D], FP32, kind="ExternalInput")
    x2_ext = nc.dram_tensor("x2", [NB, L, D], FP32, kind="ExternalInput")
    v_ext = nc.dram_tensor("att_v", [D], FP32, kind="ExternalInput")
    w_ext = nc.dram_tensor("att_W", [D, D], FP32, kind="ExternalInput")
    id_ext = nc.dram_tensor("ident", [P, P], FP16, kind="ExternalInput")
    out_ext = nc.dram_tensor("out", [NB, L, D], FP32, kind="ExternalOutput")

    ar_out = nc.dram_tensor("ar_out", [1, L], FP32, addr_space="Shared")

    with tile.TileContext(nc) as tc:
        _body(nc, tc, x_ext, x2_ext, v_ext, w_ext, id_ext, out_ext, ar_out)
    nc.compile()
    return nc


def _body(nc, tc, x_ext, x2_ext, v_ext, w_ext, id_ext, out_ext, ar_out):
    from contextlib import ExitStack

    with ExitStack() as st:
        const = st.enter_context(tc.tile_pool(name="const", bufs=1))
        rows_p = st.enter_context(tc.tile_pool(name="rows_p", bufs=1))
        cols_p = st.enter_context(tc.tile_pool(name="cols_p", bufs=1))
        scl_p = st.enter_context(tc.tile_pool(name="scl_p", bufs=1))
        cast_in = st.enter_context(tc.tile_pool(name="cast_in", bufs=4))
        xf_p = st.enter_context(tc.tile_pool(name="xf_p", bufs=2))
        cast_out = st.enter_context(tc.tile_pool(name="cast_out", bufs=2))
        x2t_p = st.enter_context(tc.tile_pool(name="x2t_p", bufs=1))
        oa_p = st.enter_context(tc.tile_pool(name="oa_p", bufs=1))
        xt_p = st.enter_context(tc.tile_pool(name="xt_p", bufs=2))
        yt_p = st.enter_context(tc.tile_pool(name="yt_p", bufs=4))
        expT_p = st.enter_context(tc.tile_pool(name="expT_p", bufs=2))
        ctx_p = st.enter_context(tc.tile_pool(name="ctx_p", bufs=2))
        rec_p = st.enter_context(tc.tile_pool(name="rec_p", bufs=2))

        # one 4-deep pool (tag "psa") serves transposes, y-psums, wlog
        # accumulators, M8 and the paired QK chunks; pc1/pc2 take the rest.
        psum_a = st.enter_context(
            tc.tile_pool(name="psum_a", bufs=4, space="PSUM"))
        psum_b = st.enter_context(
            tc.tile_pool(name="psum_b", bufs=2, space="PSUM"))
        psum_c = st.enter_context(
            tc.tile_pool(name="psum_c", bufs=2, space="PSUM"))

        dram = st.enter_context(
            tc.tile_pool(name="dram", bufs=1, space="DRAM"))

        ar_in = dram.tile([1, L], FP32, tag="ar_in")
        www = [dram.tile([1, 4 * L], FP16, tag=f"www{b}", name=f"www{b}")
               for b in range(NB)]
        xh = [dram.tile([L, D], FP16, tag=f"xh{b}", name=f"xh{b}")
              for b in range(NB)]
        ewl_d = [dram.tile([1, L], FP32, tag=f"ewld{b}", name=f"ewld{b}")
                 for b in range(NB)]

        def x_spill_stage(b, kts):
            # load fp32 x rows, cast to fp16, spill for xbar read-back.
            # (ar_in rides the scalar queue, so this chain dribbling past
            # phase A can no longer delay the collective doorbell.)
            for kt in kts:
                xf = xf_p.tile([P, D], FP32, tag="xf", name="xf")
                nc.sync.dma_start(out=xf[:], in_=x_ext[b, ts(kt, P), :])
                xc = cast_out.tile([P, D], FP16, tag="xc", name="xc")
                nc.vector.tensor_copy(xc[:], xf[:])
                nc.sync.dma_start(out=xh[b][ts(kt, P), :], in_=xc[:])

        # ---- constants (scalar DGE queue: off the bulk-load path) ----
        ident = const.tile([P, P], FP16)
        nc.scalar.dma_start(out=ident[:], in_=id_ext.ap())
        W_sb = const.tile([P, DT, D], FP16)   # W[d, e] fp16
        for dt in range(DT):
            wf = cast_in.tile([P, D], FP32, tag="cast", name="wf")
            nc.scalar.dma_start(out=wf[:], in_=w_ext[ts(dt, P), :])
            nc.scalar.copy(W_sb[:, dt, :], wf[:])
        v_sb = const.tile([P, DT], FP16)      # att_v as 6 column tiles
        vf = cast_in.tile([P, DT], FP32, tag="cast", name="vf")
        nc.scalar.dma_start(
            out=vf[:], in_=v_ext.ap().rearrange("(a p) -> p a", p=P))
        nc.scalar.copy(v_sb[:], vf[:])
        bias_sb = const.tile([P, 1], FP32)    # exp bias as a per-partition AP
        nc.vector.memset(bias_sb[:], EXP_BIAS)

        # persistent per-batch tiles
        x2T = [x2t_p.tile([P, DT, L], FP16, tag=f"x2T{b}", name=f"x2T{b}")
               for b in range(NB)]
        oa = [oa_p.tile([P, KT, D + 1], FP16, tag=f"oa{b}", name=f"oa{b}")
              for b in range(NB)]
        exp_wlog = [rows_p.tile([1, L], FP32, tag=f"ewl{b}", name=f"ewl{b}")
                    for b in range(NB)]
        eT = [cols_p.tile([16, P], FP32, tag=f"eT{b}", name=f"eT{b}")
              for b in range(NB)]

        # ---- Phase A: x2 load+cast, tensor transposes, y/wlog (qc pairs) --
        for b in range(NB):
            for qp in range(NQC // 2):
                for j in range(2 * QC // P):
                    kt = (2 * QC // P) * qp + j
                    cf = cast_in.tile([P, D], FP32, tag="cast", name="cf")
                    nc.sync.dma_start(out=cf[:], in_=x2_ext[b, ts(kt, P), :])
                    nc.vector.tensor_copy(oa[b][:, kt, 0:D], cf[:])
                    nc.vector.memset(oa[b][:, kt, D:D + 1], 1.0)
                    # x2T k-block via 6 identity-matmul transposes
                    tp = psum_a.tile([P, D], FP16, tag="psa", name="tp")
                    for dt in range(DT):
                        nc.tensor.transpose(
                            tp[:, ts(dt, P)], oa[b][:, kt, ts(dt, P)],
                            ident[:])
                    nc.vector.tensor_copy(
                        x2T[b][:, :, ts(kt, P)], tp[:].rearrange(
                            "p (dt k) -> p dt k", dt=DT))
                # y = tanh(W^T x2T) on the chunk pair, sharing W stationary
                q0, q1 = 2 * qp, 2 * qp + 1
                pw0 = psum_a.tile([1, QC], FP32, tag="psa", name="pw0")
                pw1 = psum_a.tile([1, QC], FP32, tag="psa", name="pw1")
                vready = []
                for et in range(DT):
                    psE0 = psum_a.tile([P, QC], FP32, tag="psa", name="psE0")
                    psE1 = psum_a.tile([P, QC], FP32, tag="psa", name="psE1")
                    for dt in range(DT):
                        nc.tensor.ldweights(W_sb[:, dt, ts(et, P)])
                        mm = nc.tensor.matmul(
                            psE0[:], W_sb[:, dt, ts(et, P)],
                            x2T[b][:, dt, ts(q0, QC)],
                            start=(dt == 0), stop=(dt == DT - 1))
                        mm.ins.ldweights = False
                        mm = nc.tensor.matmul(
                            psE1[:], W_sb[:, dt, ts(et, P)],
                            x2T[b][:, dt, ts(q1, QC)],
                            start=(dt == 0), stop=(dt == DT - 1))
                        mm.ins.ldweights = False
                    yt0 = yt_p.tile([P, QC], FP16, tag="yt", name="yt0")
                    yt1 = yt_p.tile([P, QC], FP16, tag="yt", name="yt1")
                    nc.scalar.activation(yt0[:], psE0[:], AF.Tanh)
                    nc.scalar.activation(yt1[:], psE1[:], AF.Tanh)
                    vready.append((et, yt0, yt1))
                    # software-pipelined wlog accumulation (one et behind)
                    if len(vready) > 1:
                        _v_mm(nc, v_sb, pw0, pw1, *vready.pop(0))
                _v_mm(nc, v_sb, pw0, pw1, *vready.pop(0))
                nc.scalar.activation(
                    exp_wlog[b][:, ts(q0, QC)], pw0[:], AF.Exp)
                nc.scalar.activation(
                    exp_wlog[b][:, ts(q1, QC)], pw1[:], AF.Exp)
            # stage exp(wlog) into 16-partition column layout (pre-AR)
            nc.scalar.dma_start(out=ewl_d[b][:], in_=exp_wlog[b][:])
            nc.scalar.dma_start(
                out=eT[b][:],
                in_=ewl_d[b][:][0, :].rearrange("(i j) -> i j", j=P))

        # batch-0 x spill: queued after all phase-A loads, so it executes
        # in the phase-A tail and the AllReduce bubble where DMA is idle.
        # It must finish before attention(0)'s xbar read-back of xh[0].
        x_spill_stage(0, range(KT))

        # ---- AllReduce of sum_b exp(wlog) over the 8 cores ----
        pc_t = cols_p.tile([16, P], FP32, tag="pcols")
        nc.vector.tensor_add(pc_t[:], eT[0][:], eT[1][:])
        nc.scalar.dma_start(out=ar_in[:], in_=pc_t[:])
        nc.gpsimd.collective_compute(
            "AllReduce", mybir.AluOpType.add,
            replica_groups=[list(range(NCORES))],
            ins=[ar_in[:].opt()], outs=[ar_out.ap().opt()])

        # ---- w = exp_wlog/denom in [16,128] column-parallel layout ----
        dT = cols_p.tile([16, P], FP32, tag="dT")
        nc.scalar.dma_start(
            out=dT[:], in_=ar_out.ap()[0, :].rearrange("(i j) -> i j", j=P))
        nc.vector.reciprocal(dT[:], dT[:])
        wq16 = [scl_p.tile([P, D], FP16, tag=f"wq{b}", name=f"wq{b}")
                for b in range(NB)]
        m8h = [scl_p.tile([P, DT, 8], FP16, tag=f"m8h{b}", name=f"m8h{b}")
               for b in range(NB)]
        w16c = cols_p.tile([16, P], FP16, tag="w16c")

        def scale_block(b):
            # w[b] columns (fp16); write w 4x contiguously into DRAM
            nc.vector.tensor_mul(eT[b][:], eT[b][:], dT[:])
            nc.vector.tensor_copy(w16c[:], eT[b][:])
            for k in range(4):
                nc.scalar.dma_start(out=www[b][:, ts(k, L)], in_=w16c[:])
            # wq[p,j] = w[(768*(p%8)+j)%L] -- 16 copies of an [8,768] read
            wq8 = www[b][:][0, 0:6144].rearrange("(r j) -> r j", j=D)
            nc.scalar.dma_start(out=wq16[b][0:8, :], in_=wq8)
            # M8[p,dt,r] = wq[r, 128dt+p]: transpose [8,128] slices of wq
            tp2 = psum_a.tile([P, D], FP16, tag="psa", name="tp2")
            for dt in range(DT):
                nc.tensor.transpose(
                    tp2[:, dt * 8:(dt + 1) * 8], wq16[b][0:8, ts(dt, P)],
                    ident[0:8, 0:8])
            nc.vector.tensor_copy(
                m8h[b][:], tp2[:, 0:DT * 8].rearrange(
                    "p (dt r) -> p dt r", dt=DT))
            # x2T := outT first (it gates QK); M8 broadcast along k//8
            for dt in range(DT):
                nc.vector.tensor_mul(
                    x2T[b][:, dt, :].rearrange("p (m r) -> p m r", r=8),
                    x2T[b][:, dt, :].rearrange("p (m r) -> p m r", r=8),
                    m8h[b][:, dt, :].unsqueeze(1).broadcast_to(
                        [P, L // 8, 8]))
            # remaining wq partitions, then oa := out (natural)
            for a in range(1, 16):
                nc.scalar.dma_start(out=wq16[b][ts(a, 8), :], in_=wq8)
            for kt in range(KT):
                nc.vector.tensor_mul(
                    oa[b][:, kt, 0:D], oa[b][:, kt, 0:D], wq16[b][:])

        def attention(b):
            for qp in range(NQC // 2):
                q0, q1 = 2 * qp, 2 * qp + 1
                xt0 = xt_p.tile([P, DT, QC], FP16, tag="xt", name="xt0")
                xt1 = xt_p.tile([P, DT, QC], FP16, tag="xt", name="xt1")
                for dt in range(DT):
                    nc.sync.dma_start_transpose(
                        xt0[:, dt, :], xh[b][ts(q0, QC), ts(dt, P)])
                    nc.sync.dma_start_transpose(
                        xt1[:, dt, :], xh[b][ts(q1, QC), ts(dt, P)])
                if b == 0:
                    # batch-1 x spill rides batch-0's idle attention DMA
                    # (emitted after this pair's transposes in queue order)
                    x_spill_stage(1, range(8 * qp, 8 * qp + 8))
                expT0 = expT_p.tile([P, KT, QC], FP16, tag="expT",
                                    name="expT0")
                expT1 = expT_p.tile([P, KT, QC], FP16, tag="expT",
                                    name="expT1")
                for kt in range(KT):
                    ps0 = psum_a.tile([P, QC], FP32, tag="psa", name="ps_qk0")
                    ps1 = psum_a.tile([P, QC], FP32, tag="psa", name="ps_qk1")
                    for dt in range(DT):
                        nc.tensor.ldweights(x2T[b][:, dt, ts(kt, P)])
                        mm = nc.tensor.matmul(
                            ps0[:], x2T[b][:, dt, ts(kt, P)], xt0[:, dt, :],
                            start=(dt == 0), stop=(dt == DT - 1))
                        mm.ins.ldweights = False
                        mm = nc.tensor.matmul(
                            ps1[:], x2T[b][:, dt, ts(kt, P)], xt1[:, dt, :],
                            start=(dt == 0), stop=(dt == DT - 1))
                        mm.ins.ldweights = False
                    nc.scalar.activation(expT0[:, kt, :], ps0[:], AF.Exp,
                                         bias=bias_sb[:])
                    nc.scalar.activation(expT1[:, kt, :], ps1[:], AF.Exp,
                                         bias=bias_sb[:])
                for qq, expT in ((q0, expT0), (q1, expT1)):
                    for qt in range(QC // P):
                        pc1 = psum_b.tile([P, 512], FP32, tag="psb",
                                          name="pc1")
                        pc2 = psum_c.tile([P, 257], FP32, tag="psc",
                                          name="pc2")
                        for kt in range(KT):
                            lh = expT[:, kt, ts(qt, P)]
                            nc.tensor.ldweights(lh)
                            mm = nc.tensor.matmul(
                                pc1[:], lh, oa[b][:, kt, 0:512],
                                start=(kt == 0), stop=(kt == KT - 1))
                            mm.ins.ldweights = False
                            mm = nc.tensor.matmul(
                                pc2[:], lh, oa[b][:, kt, 512:D + 1],
                                start=(kt == 0), stop=(kt == KT - 1))
                            mm.ins.ldweights = False
                        rec = rec_p.tile([P, 1], FP32, name="rec")
                        nc.vector.reciprocal(rec[:], pc2[:, 256:257])
                        cc = ctx_p.tile([P, D], FP32, tag="cc", name="cc")
                        nc.vector.tensor_scalar_mul(
                            cc[:, 0:512], pc1[:], rec[:])
                        nc.vector.tensor_scalar_mul(
                            cc[:, 512:D], pc2[:, 0:256], rec[:])
                        q_0 = qq * QC + qt * P
                        nc.scalar.dma_start(
                            out=out_ext[b, q_0:q_0 + P, :], in_=cc[:])

        # batch-1 scale work is emitted after batch-0's attention so its
        # dependencies can never stall batch-0's QK start.
        scale_block(0)
        attention(0)
        scale_block(1)
        attention(1)


def _v_mm(nc, v_sb, pw0, pw1, et, yt0, yt1):
    nc.tensor.ldweights(v_sb[:, et:et + 1])
    mm = nc.tensor.matmul(pw0[:], v_sb[:, et:et + 1], yt0[:],
                          start=(et == 0), stop=(et == DT - 1))
    mm.ins.ldweights = False
    mm = nc.tensor.matmul(pw1[:], v_sb[:, et:et + 1], yt1[:],
                          start=(et == 0), stop=(et == DT - 1))
    mm.ins.ldweights = False


_NC_CACHE = None


def kernel(x, x2, att_v, att_W):
    global _NC_CACHE
    if _NC_CACHE is None:
        _NC_CACHE = build_nc()
    nc = _NC_CACHE

    x = np.ascontiguousarray(x, dtype=np.float32)
    x2 = np.ascontiguousarray(x2, dtype=np.float32)
    att_v = np.ascontiguousarray(att_v, dtype=np.float32)
    att_W = np.ascontiguousarray(att_W, dtype=np.float32)

    ident = np.eye(P, dtype=np.float16)
    in_maps = []
    for i in range(NCORES):
        sl = slice(i * NB, (i + 1) * NB)
        in_maps.append({
            "x": x[sl], "x2": x2[sl], "att_v": att_v, "att_W": att_W,
            "ident": ident,
        })
    res = run_bass_kernel_spmd(nc, in_maps, core_ids=list(range(NCORES)))
    outs = [res.results[i]["out"] for i in range(NCORES)]
    return np.concatenate(outs, axis=0).astype(np.float32)


if __name__ == "__main__":
    xs = np.random.randn(B, L, D).astype(np.float32)
    x2s = np.random.randn(B, L, D).astype(np.float32)
    vs = (np.random.randn(D) * 0.01).astype(np.float32)
    Ws = (np.random.randn(D, D) * 0.01).astype(np.float32)
    o = kernel(x=xs, x2=x2s, att_v=vs, att_W=Ws)
    print(o.shape, o.dtype)


# revision 64
# speedup vs baseline: 1.1845x; 1.1845x over previous
"""Trainium2 Bass kernel for nn_Aspect_Attention_op2 (B=16, L=2048, D=768).

reference semantics:
    y = tanh(x2 @ att_W)                        # [B, L, D]
    wlog = einsum('d,bld->bl', att_v, y)        # [B, L]
    w = softmax(wlog, axis=0)                   # softmax over BATCH
    w_tiled[b,i,j] = w[b, (i*D+j) % L]          # tile-then-reshape (windowed!)
    out = x2 * w_tiled
    score = x @ out^T ; attn = softmax(score, -1) ; ctx = attn @ out

Distribution: batch-parallel, 2 batches/core on 8 cores. The batch softmax
needs one 8KB AllReduce(add) of sum_b exp(wlog) (max-subtraction is skipped:
wlog absmax ~0.4, score absmax ~13; the attention exp carries a -4 bias so
exp(score-4) fits fp16, which cancels in the softmax ratio).

Key structure (all operands fp16, psum fp32; measured rel err ~7e-4):
  * The window multiplier w[(i*D+j) % L] is periodic with period 8 in the
    row index, so `out` never exists in DRAM:
      - natural: out[:,kt,j] = x2[:,kt,j] * wq[p,j] with one fixed [128,768]
        tile wq[p,j] = w[(768*(p%8)+j)%L] for ALL kt, applied in place to
        the SBUF-resident fp16 x2 copy (which carries the PV ones-column
        that yields the softmax denominators).
      - transposed: outT[p,dt,k] = x2T[p,dt,k] * M8[p,dt,k%8] with
        M8[p,dt,r] = w[(768r+128dt+p)%L], applied in place to x2T via a
        stride-0 broadcast along k//8.  M8 itself is a tensor-engine
        transpose of wq's first 8 partitions.
    wq comes from affine DMA reads of a 4x-replicated copy of w in DRAM
    (offsets 768r+j < 8192 need no modulo).
  * x2T is produced by tensor-engine transposes (identity matmul, identity
    shipped as a constant input) straight from the SBUF cast tiles -- no
    DRAM spill / xbar read-back for x2.
  * xT is read back with the DMA xbar transpose from an fp16 spill of x;
    the spill runs off the critical path (batch 0 spread through phase A,
    batch 1 inside batch 0's attention where DMA is idle).
  * q-chunks are processed in PAIRS sharing each stationary tile: the
    second matmul of each pair sets ldweights=False so the PE reuses the
    loaded weights (halves LDWEIGHTS in QK/y/v; PV's two output slices
    share the loaded attn tile the same way).
  * All w/softmax bookkeeping runs in [16,128] column-parallel layout
    (128x faster than single-partition row ops); batch 1's post-AllReduce
    scaling is emitted after batch 0's attention so it cannot block it.

NOTE: gpsimd must run ONLY the collective -- any other gpsimd instruction
ahead of it perturbs the TOPSP doorbell and adds ~2.5ms to the AllReduce.
"""

import sys

try:
    import concourse  # noqa: F401
except ImportError:
    sys.path.insert(0, "/opt/trn_rl_repo")

import numpy as np

import concourse.bass as bass
import concourse.bacc as bacc
import concourse.mybir as mybir
import concourse.tile as tile
from concourse.bass_utils import run_bass_kernel_spmd

B, L, D = 16, 2048, 768
NCORES = 8
NB = B // NCORES          # batches per core = 2
P = 128
DT = D // P               # 6 d-tiles
KT = L // P               # 16 k-tiles
QC = 512                  # q-chunk (psum free dim)
NQC = L // QC             # 4 q-chunks
FP32 = mybir.dt.float32
FP16 = mybir.dt.float16
AF = mybir.ActivationFunctionType
EXP_BIAS = -4.0           # exp(score-4) <= ~5e3 fits fp16; cancels in ratio


def ts(i, n):
    return bass.ts(i, n)


def build_nc():
    nc = bacc.Bacc("TRN2", target_bir_lowering=False, debug=False,
                   num_devices=NCORES)

    x_ext = nc.dram_tensor("x", [NB, L, D], FP32, kind="ExternalInput")
    x2_ext = nc.dram_tensor("x2", [NB, L, D], FP32, kind="ExternalInput")
    v_ext = nc.dram_tensor("att_v", [D], FP32, kind="ExternalInput")
    w_ext = nc.dram_tensor("att_W", [D, D], FP32, kind="ExternalInput")
    id_ext = nc.dram_tensor("ident", [P, P], FP16, kind="ExternalInput")
    out_ext = nc.dram_tensor("out", [NB, L, D], FP32, kind="ExternalOutput")

    ar_out = nc.dram_tensor("ar_out", [1, L], FP32, addr_space="Shared")

    with tile.TileContext(nc) as tc:
        _body(nc, tc, x_ext, x2_ext, v_ext, w_ext, id_ext, out_ext, ar_out)
    nc.compile()
    return nc


def _body(nc, tc, x_ext, x2_ext, v_ext, w_ext, id_ext, out_ext, ar_out):
    from contextlib import ExitStack

    with ExitStack() as st:
        const = st.enter_context(tc.tile_pool(name="const", bufs=1))
        rows_p = st.enter_context(tc.tile_pool(name="rows_p", bufs=1))
        cols_p = st.enter_context(tc.tile_pool(name="cols_p", bufs=1))
        scl_p = st.enter_context(tc.tile_pool(name="scl_p", bufs=1))
        cast_in = st.enter_context(tc.tile_pool(name="cast_in", bufs=4))
        xf_p = st.enter_context(tc.tile_pool(name="xf_p", bufs=2))
        cast_out = st.enter_context(tc.tile_pool(name="cast_out", bufs=2))
        x2t_p = st.enter_context(tc.tile_pool(name="x2t_p", bufs=1))
        oa_p = st.enter_context(tc.tile_pool(name="oa_p", bufs=1))
        xt_p = st.enter_context(tc.tile_pool(name="xt_p", bufs=2))
        yt_p = st.enter_context(tc.tile_pool(name="yt_p", bufs=4))
        expT_p = st.enter_context(tc.tile_pool(name="expT_p", bufs=2))
        ctx_p = st.enter_context(tc.tile_pool(name="ctx_p", bufs=2))
        rec_p = st.enter_context(tc.tile_pool(name="rec_p", bufs=2))

        # one 4-deep pool (tag "psa") serves transposes, y-psums, wlog
        # accumulators, M8 and the paired QK chunks; pc1/pc2 take the rest.
        psum_a = st.enter_context(
            tc.tile_pool(name="psum_a", bufs=4, space="PSUM"))
        psum_b = st.enter_context(
            tc.tile_pool(name="psum_b", bufs=2, space="PSUM"))
        psum_c = st.enter_context(
            tc.tile_pool(name="psum_c", bufs=2, space="PSUM"))

        dram = st.enter_context(
            tc.tile_pool(name="dram", bufs=1, space="DRAM"))

        ar_in = dram.tile([1, L], FP32, tag="ar_in")
        www = [dram.tile([1, 4 * L], FP16, tag=f"www{b}", name=f"www{b}")
               for b in range(NB)]
        xh = [dram.tile([L, D], FP16, tag=f"xh{b}", name=f"xh{b}")
              for b in range(NB)]
        ewl_d = [dram.tile([1, L], FP32, tag=f"ewld{b}", name=f"ewld{b}")
                 for b in range(NB)]

        def x_spill_stage(b, kts):
            # load fp32 x rows, cast to fp16, spill for xbar read-back.
            # (ar_in rides the scalar queue, so this chain dribbling past
            # phase A can no longer delay the collective doorbell.)
            for kt in kts:
                xf = xf_p.tile([P, D], FP32, tag="xf", name="xf")
                nc.sync.dma_start(out=xf[:], in_=x_ext[b, ts(kt, P), :])
                xc = cast_out.tile([P, D], FP16, tag="xc", name="xc")
                nc.vector.tensor_copy(xc[:], xf[:])
                nc.sync.dma_start(out=xh[b][ts(kt, P), :], in_=xc[:])

        # ---- constants (scalar DGE queue: off the bulk-load path) ----
        ident = const.tile([P, P], FP16)
        nc.scalar.dma_start(out=ident[:], in_=id_ext.ap())
        W_sb = const.tile([P, DT, D], FP16)   # W[d, e] fp16
        for dt in range(DT):
            wf = cast_in.tile([P, D], FP32, tag="cast", name="wf")
            nc.scalar.dma_start(out=wf[:], in_=w_ext[ts(dt, P), :])
            nc.scalar.copy(W_sb[:, dt, :], wf[:])
        v_sb = const.tile([P, DT], FP16)      # att_v as 6 column tiles
        vf = cast_in.tile([P, DT], FP32, tag="cast", name="vf")
        nc.scalar.dma_start(
            out=vf[:], in_=v_ext.ap().rearrange("(a p) -> p a", p=P))
        nc.scalar.copy(v_sb[:], vf[:])
        bias_sb = const.tile([P, 1], FP32)    # exp bias as a per-partition AP
        nc.vector.memset(bias_sb[:], EXP_BIAS)

        # persistent per-batch tiles
        x2T = [x2t_p.tile([P, DT, L], FP16, tag=f"x2T{b}", name=f"x2T{b}")
               for b in range(NB)]
        oa = [oa_p.tile([P, KT, D + 1], FP16, tag=f"oa{b}", name=f"oa{b}")
              for b in range(NB)]
        exp_wlog = [rows_p.tile([1, L], FP32, tag=f"ewl{b}", name=f"ewl{b}")
                    for b in range(NB)]
        eT = [cols_p.tile([16, P], FP32, tag=f"eT{b}", name=f"eT{b}")
              for b in range(NB)]

        # ---- Phase A: x2 load+cast, tensor transposes, y/wlog (qc pairs) --
        for b in range(NB):
            for qp in range(NQC // 2):
                for j in range(2 * QC // P):
                    kt = (2 * QC // P) * qp + j
                    cf = cast_in.tile([P, D], FP32, tag="cast", name="cf")
                    nc.sync.dma_start(out=cf[:], in_=x2_ext[b, ts(kt, P), :])
                    nc.vector.tensor_copy(oa[b][:, kt, 0:D], cf[:])
                    nc.vector.memset(oa[b][:, kt, D:D + 1], 1.0)
                    # x2T k-block via 6 identity-matmul transposes
                    tp = psum_a.tile([P, D], FP16, tag="psa", name="tp")
                    for dt in range(DT):
                        nc.tensor.transpose(
                            tp[:, ts(dt, P)], oa[b][:, kt, ts(dt, P)],
                            ident[:])
                    nc.vector.tensor_copy(
                        x2T[b][:, :, ts(kt, P)], tp[:].rearrange(
                            "p (dt k) -> p dt k", dt=DT))
                # y = tanh(W^T x2T) on the chunk pair, sharing W stationary
                q0, q1 = 2 * qp, 2 * qp + 1
                pw0 = psum_a.tile([1, QC], FP32, tag="psa", name="pw0")
                pw1 = psum_a.tile([1, QC], FP32, tag="psa", name="pw1")
                vready = []
                for et in range(DT):
                    psE0 = psum_a.tile([P, QC], FP32, tag="psa", name="psE0")
                    psE1 = psum_a.tile([P, QC], FP32, tag="psa", name="psE1")
                    for dt in range(DT):
                        nc.tensor.matmul(
                            psE0[:], W_sb[:, dt, ts(et, P)],
                            x2T[b][:, dt, ts(q0, QC)],
                            start=(dt == 0), stop=(dt == DT - 1))
                        mm = nc.tensor.matmul(
                            psE1[:], W_sb[:, dt, ts(et, P)],
                            x2T[b][:, dt, ts(q1, QC)],
                            start=(dt == 0), stop=(dt == DT - 1))
                        mm.ins.ldweights = False
                    yt0 = yt_p.tile([P, QC], FP16, tag="yt", name="yt0")
                    yt1 = yt_p.tile([P, QC], FP16, tag="yt", name="yt1")
                    nc.scalar.activation(yt0[:], psE0[:], AF.Tanh)
                    nc.scalar.activation(yt1[:], psE1[:], AF.Tanh)
                    vready.append((et, yt0, yt1))
                    # software-pipelined wlog accumulation (one et behind)
                    if len(vready) > 1:
                        _v_mm(nc, v_sb, pw0, pw1, *vready.pop(0))
                _v_mm(nc, v_sb, pw0, pw1, *vready.pop(0))
                nc.scalar.activation(
                    exp_wlog[b][:, ts(q0, QC)], pw0[:], AF.Exp)
                nc.scalar.activation(
                    exp_wlog[b][:, ts(q1, QC)], pw1[:], AF.Exp)
            # stage exp(wlog) into 16-partition column layout (pre-AR)
            nc.scalar.dma_start(out=ewl_d[b][:], in_=exp_wlog[b][:])
            nc.scalar.dma_start(
                out=eT[b][:],
                in_=ewl_d[b][:][0, :].rearrange("(i j) -> i j", j=P))

        # batch-0 x spill: queued after all phase-A loads, so it executes
        # in the phase-A tail and the AllReduce bubble where DMA is idle.
        # It must finish before attention(0)'s xbar read-back of xh[0].
        x_spill_stage(0, range(KT))

        # ---- AllReduce of sum_b exp(wlog) over the 8 cores ----
        pc_t = cols_p.tile([16, P], FP32, tag="pcols")
        nc.vector.tensor_add(pc_t[:], eT[0][:], eT[1][:])
        nc.scalar.dma_start(out=ar_in[:], in_=pc_t[:])
        nc.gpsimd.collective_compute(
            "AllReduce", mybir.AluOpType.add,
            replica_groups=[list(range(NCORES))],
            ins=[ar_in[:].opt()], outs=[ar_out.ap().opt()])

        # ---- w = exp_wlog/denom in [16,128] column-parallel layout ----
        dT = cols_p.tile([16, P], FP32, tag="dT")
        nc.scalar.dma_start(
            out=dT[:], in_=ar_out.ap()[0, :].rearrange("(i j) -> i j", j=P))
        nc.vector.reciprocal(dT[:], dT[:])
        wq16 = [scl_p.tile([P, D], FP16, tag=f"wq{b}", name=f"wq{b}")
                for b in range(NB)]
        m8h = [scl_p.tile([P, DT, 8], FP16, tag=f"m8h{b}", name=f"m8h{b}")
               for b in range(NB)]
        w16c = cols_p.tile([16, P], FP16, tag="w16c")

        def scale_block(b):
            # w[b] columns (fp16); write w 4x contiguously into DRAM
            nc.vector.tensor_mul(eT[b][:], eT[b][:], dT[:])
            nc.vector.tensor_copy(w16c[:], eT[b][:])
            for k in range(4):
                nc.scalar.dma_start(out=www[b][:, ts(k, L)], in_=w16c[:])
            # wq[p,j] = w[(768*(p%8)+j)%L] -- 16 copies of an [8,768] read
            wq8 = www[b][:][0, 0:6144].rearrange("(r j) -> r j", j=D)
            nc.scalar.dma_start(out=wq16[b][0:8, :], in_=wq8)
            # M8[p,dt,r] = wq[r, 128dt+p]: transpose [8,128] slices of wq
            tp2 = psum_a.tile([P, D], FP16, tag="psa", name="tp2")
            for dt in range(DT):
                nc.tensor.transpose(
                    tp2[:, dt * 8:(dt + 1) * 8], wq16[b][0:8, ts(dt, P)],
                    ident[0:8, 0:8])
            nc.vector.tensor_copy(
                m8h[b][:], tp2[:, 0:DT * 8].rearrange(
                    "p (dt r) -> p dt r", dt=DT))
            # x2T := outT first (it gates QK); M8 broadcast along k//8
            for dt in range(DT):
                nc.vector.tensor_mul(
                    x2T[b][:, dt, :].rearrange("p (m r) -> p m r", r=8),
                    x2T[b][:, dt, :].rearrange("p (m r) -> p m r", r=8),
                    m8h[b][:, dt, :].unsqueeze(1).broadcast_to(
                        [P, L // 8, 8]))
            # remaining wq partitions, then oa := out (natural)
            for a in range(1, 16):
                nc.scalar.dma_start(out=wq16[b][ts(a, 8), :], in_=wq8)
            for kt in range(KT):
                nc.vector.tensor_mul(
                    oa[b][:, kt, 0:D], oa[b][:, kt, 0:D], wq16[b][:])

        def attention(b):
            for qp in range(NQC // 2):
                q0, q1 = 2 * qp, 2 * qp + 1
                xt0 = xt_p.tile([P, DT, QC], FP16, tag="xt", name="xt0")
                xt1 = xt_p.tile([P, DT, QC], FP16, tag="xt", name="xt1")
                for dt in range(DT):
                    nc.sync.dma_start_transpose(
                        xt0[:, dt, :], xh[b][ts(q0, QC), ts(dt, P)])
                    nc.sync.dma_start_transpose(
                        xt1[:, dt, :], xh[b][ts(q1, QC), ts(dt, P)])
                if b == 0:
                    # batch-1 x spill rides batch-0's idle attention DMA
                    # (emitted after this pair's transposes in queue order)
                    x_spill_stage(1, range(8 * qp, 8 * qp + 8))
                expT0 = expT_p.tile([P, KT, QC], FP16, tag="expT",
                                    name="expT0")
                expT1 = expT_p.tile([P, KT, QC], FP16, tag="expT",
                                    name="expT1")
                for kt in range(KT):
                    ps0 = psum_a.tile([P, QC], FP32, tag="psa", name="ps_qk0")
                    ps1 = psum_a.tile([P, QC], FP32, tag="psa", name="ps_qk1")
                    for dt in range(DT):
                        nc.tensor.matmul(
                            ps0[:], x2T[b][:, dt, ts(kt, P)], xt0[:, dt, :],
                            start=(dt == 0), stop=(dt == DT - 1))
                        mm = nc.tensor.matmul(
                            ps1[:], x2T[b][:, dt, ts(kt, P)], xt1[:, dt, :],
                            start=(dt == 0), stop=(dt == DT - 1))
                        mm.ins.ldweights = False
                    nc.scalar.activation(expT0[:, kt, :], ps0[:], AF.Exp,
                                         bias=bias_sb[:])
                    nc.scalar.activation(expT1[:, kt, :], ps1[:], AF.Exp,
                                         bias=bias_sb[:])
                for qq, expT in ((q0, expT0), (q1, expT1)):
                    for qt in range(QC // P):
                        pc1 = psum_b.tile([P, 512], FP32, tag="psb",
                                          name="pc1")
                        pc2 = psum_c.tile([P, 257], FP32, tag="psc",
                                          name="pc2")
                        for kt in range(KT):
                            lh = expT[:, kt, ts(qt, P)]
                            nc.tensor.matmul(
                                pc1[:], lh, oa[b][:, kt, 0:512],
                                start=(kt == 0), stop=(kt == KT - 1))
                            mm = nc.tensor.matmul(
                                pc2[:], lh, oa[b][:, kt, 512:D + 1],
                                start=(kt == 0), stop=(kt == KT - 1))
                            mm.ins.ldweights = False
                        rec = rec_p.tile([P, 1], FP32, name="rec")
                        nc.vector.reciprocal(rec[:], pc2[:, 256:257])
                        cc = ctx_p.tile([P, D], FP32, tag="cc", name="cc")
                        nc.vector.tensor_scalar_mul(
                            cc[:, 0:512], pc1[:], rec[:])
                        nc.vector.tensor_scalar_mul(
                            cc[:, 512:D], pc2[:, 0:256], rec[:])
                        q_0 = qq * QC + qt * P
                        nc.scalar.dma_start(
                            out=out_ext[b, q_0:q_0 + P, :], in_=cc[:])

        # batch-1 scale work is emitted after batch-0's attention so its
        # dependencies can never stall batch-0's QK start.
        scale_block(0)
        attention(0)
        scale_block(1)
        attention(1)


def _v_mm(nc, v_sb, pw0, pw1, et, yt0, yt1):
    nc.tensor.matmul(pw0[:], v_sb[:, et:et + 1], yt0[:],
                     start=(et == 0), stop=(et == DT - 1))
    mm = nc.tensor.matmul(pw1[:], v_sb[:, et:et + 1], yt1[:],
                          start=(et == 0), stop=(et == DT - 1))
    mm.ins.ldweights = False


_NC_CACHE = None


def kernel(x, x2, att_v, att_W):
    global _NC_CACHE
    if _NC_CACHE is None:
        _NC_CACHE = build_nc()
    nc = _NC_CACHE

    x = np.ascontiguousarray(x, dtype=np.float32)
    x2 = np.ascontiguousarray(x2, dtype=np.float32)
    att_v = np.ascontiguousarray(att_v, dtype=np.float32)
    att_W = np.ascontiguousarray(att_W, dtype=np.float32)

    ident = np.eye(P, dtype=np.float16)
    in_maps = []
    for i in range(NCORES):
        sl = slice(i * NB, (i + 1) * NB)
        in_maps.append({
            "x": x[sl], "x2": x2[sl], "att_v": att_v, "att_W": att_W,
            "ident": ident,
        })
    res = run_bass_kernel_spmd(nc, in_maps, core_ids=list(range(NCORES)))
    outs = [res.results[i]["out"] for i in range(NCORES)]
    return np.concatenate(outs, axis=0).astype(np.float32)


if __name__ == "__main__":
    xs = np.random.randn(B, L, D).astype(np.float32)
    x2s = np.random.randn(B, L, D).astype(np.float32)
    vs = (np.random.randn(D) * 0.01).astype(np.float32)
    Ws = (np.random.randn(D, D) * 0.01).astype(np.float32)
    o = kernel(x=xs, x2=x2s, att_v=vs, att_W=Ws)
    print(o.shape, o.dtype)
